# revision 51
# baseline (speedup 1.0000x reference)
"""Multi-head attention (B=4, S=2048, D=1024, H=16) on 8 Trainium2 cores.

Sharding: data-parallel over batch (4) x tensor-parallel over heads (2).
Core c handles batch c//2 and heads (c%2)*8 .. +8.  Each core computes a
partial output (its heads' contribution through the O-projection); the host
sums the two partials per batch and adds the output bias.

v3 schedule: the attention inner loop is scalar-engine-bound (exp), so all
other PE work is software-pipelined into it.  Q/K are projected one
head-pair at a time, with head-pair pc+1's projection matmuls interleaved
as filler into head-pair pc's attention window; the O-projection is
interleaved into the last head-pair's window.  Softmax normalization uses
same-partition copies of the in-AV denominator rows, one batched DVE
reciprocal, and GPSIMD partition-broadcasts (no DRAM round trips).
"""

import numpy as np
from collections import deque
from contextlib import ExitStack

import ml_dtypes
import concourse.bass as bass
import concourse.tile as tile
from concourse import bacc, library_config, mybir
from concourse.bass import ts
from concourse.bass_utils import run_bass_kernel_spmd

P = 128
S = 2048          # sequence length
D = 1024          # model dim
DOUT = 512        # per-core projection width (8 heads x 64)
DK = 64           # head dim
B = 4
N_CORES = 8
F32 = mybir.dt.float32
BF16 = mybir.dt.bfloat16
FP = mybir.ActivationFunctionType

NKC = D // P      # 8 contraction chunks over model dim
NPC = DOUT // P   # 4 head pairs per core
NQ = S // 512     # 4 query chunks of 512
NKI = S // P      # 16 key chunks of 128

_cached_nc = None


def _emit(ctx: ExitStack, tc: "tile.TileContext", io: dict):
    nc = tc.nc

    qt_r = io["qt"].ap().rearrange("(c p) s -> p c s", p=P)      # [128, 8, 2048]
    kt_r = io["kt"].ap().rearrange("(c p) s -> p c s", p=P)
    vt_r = io["vt"].ap().rearrange("(c p) s -> p c s", p=P)
    wqt_r = io["wqt"].ap().rearrange("(c p) m -> p c m", p=P)    # [128, 8, 512]
    wkt_r = io["wkt"].ap().rearrange("(c p) m -> p c m", p=P)
    wvt_r = io["wvt"].ap().rearrange("(c p) m -> p c m", p=P)
    wot_r = io["wot"].ap().rearrange("(c p) n -> p c n", p=P)    # [128, 4, 1024]
    bq_r = io["bq"].ap().rearrange("(c p) -> p c", p=P)          # [128, 4]
    bk_r = io["bk"].ap().rearrange("(c p) -> p c", p=P)
    bv_ap = io["bv"].ap()                                        # [512]
    out_r = io["out"].ap().rearrange("(sc p) n -> p sc n", p=P)  # [128, 16, 1024]

    persist = ctx.enter_context(tc.tile_pool(name="persist", bufs=1))
    weights = ctx.enter_context(tc.tile_pool(name="weights", bufs=2))
    streams = ctx.enter_context(tc.tile_pool(name="streams", bufs=4))
    streams2 = ctx.enter_context(tc.tile_pool(name="streams2", bufs=4))
    vstream = ctx.enter_context(tc.tile_pool(name="vstream", bufs=3))
    etp = ctx.enter_context(tc.tile_pool(name="etp", bufs=4))
    avsb = ctx.enter_context(tc.tile_pool(name="avsb", bufs=2))
    denp = ctx.enter_context(tc.tile_pool(name="denp", bufs=2))
    repp = ctx.enter_context(tc.tile_pool(name="repp", bufs=2))
    stagp = ctx.enter_context(tc.tile_pool(name="stagp", bufs=2))
    outp = ctx.enter_context(tc.tile_pool(name="outp", bufs=3))

    # PSUM: st 2x[128,1024] (4 banks) + av 2x[128,512] (2) + proj 2x[128,512] (2)
    ps_st = ctx.enter_context(tc.tile_pool(name="ps_st", bufs=2, space="PSUM"))
    ps_av = ctx.enter_context(tc.tile_pool(name="ps_av", bufs=2, space="PSUM"))
    ps_proj = ctx.enter_context(tc.tile_pool(name="ps_proj", bufs=2, space="PSUM"))

    # ---- constants / biases ------------------------------------------------
    # wv + first V input chunk gate the first matmul: wv rides the idle
    # scalar queue in parallel with vt chunks on sync, both split into
    # separate kc-half tiles (dependency tracking is tile-granular) so the
    # kc=0..3 matmuls can start while the second halves stream in
    wv_h = []
    for q in range(2):
        t = persist.tile([P, 4, DOUT], BF16, tag=f"wvh{q}")
        nc.scalar.dma_start(out=t, in_=wvt_r[:, 4 * q : 4 * q + 4, :])
        wv_h.append(t)
    # bv replicated across partitions (DMA partition-broadcast, stride 0)
    bv_rep = persist.tile([P, DOUT], F32, tag="bvrep")
    bv_bcast = bass.AP(
        tensor=bv_ap.tensor, offset=bv_ap.offset, ap=[[0, P]] + list(bv_ap.ap)
    )
    nc.gpsimd.dma_start(out=bv_rep, in_=bv_bcast)
    # gpsimd ucode library with InstPartitionBroadcast (standard lacks it);
    # emitted after the bv DMA so it doesn't gate the V projection
    nc.gpsimd.load_library(library_config.attn)
    bq_sb = persist.tile([P, NPC], F32, tag="bq")
    nc.gpsimd.dma_start(out=bq_sb, in_=bq_r)
    bk_sb = persist.tile([P, NPC], F32, tag="bk")
    nc.gpsimd.dma_start(out=bk_sb, in_=bk_r)
    wo_sb = persist.tile([P, NPC, D], BF16, tag="wo")
    nc.gpsimd.dma_start(out=wo_sb, in_=wot_r)

    # ---- persistent activations (bf16) --------------------------------------
    # qT/kT double-buffered across head pairs, split into four 512-col tiles
    # each: dependency tracking is tile-granular, so per-chunk tiles let the
    # next head pair's first scores start before its last chunk is projected
    qTb = [
        [
            persist.tile([P, 512], BF16, tag=f"qT{i}_{s}", name=f"qT{i}_{s}")
            for s in range(NQ)
        ]
        for i in range(2)
    ]
    kTb = [
        [
            persist.tile([P, 512], BF16, tag=f"kT{i}_{s}", name=f"kT{i}_{s}")
            for s in range(NQ)
        ]
        for i in range(2)
    ]
    # v: [s, head, dk+1]; col 64 of each head block holds ones so the AV
    # matmul's 65th output row accumulates the softmax denominator
    v_sb = [
        persist.tile([P, 8, 65], BF16, tag=f"v{i}", name=f"v{i}") for i in range(NKI)
    ]
    for i in range(NKI):
        nc.vector.memset(v_sb[i][:, :, 64:65], 1.0)
    # attn_outT: [dout, s] as 4x4 tiles of [128, 512] (rows 0-63 even head,
    # 64-127 odd; per-qi-block tiles keep O-projection dependencies exact)
    aoT = [
        [
            persist.tile([P, 512], BF16, tag=f"aoT{m}_{s}", name=f"aoT{m}_{s}")
            for s in range(NQ)
        ]
        for m in range(NPC)
    ]

    # ---- filler generators --------------------------------------------------
    def gen_qk_proj(pc, prefix=False):
        """Project qT/kT for head pair pc.  Each yield is ~1 PE slot."""
        wq_sb = weights.tile([P, NKC, P], BF16, tag="wq")
        nc.sync.dma_start(out=wq_sb, in_=wqt_r[:, :, ts(pc, P)])
        wk_sb = weights.tile([P, NKC, P], BF16, tag="wk")
        nc.sync.dma_start(out=wk_sb, in_=wkt_r[:, :, ts(pc, P)])
        yield
        # interleaved prefetch of input chunks; in the prefix sync still
        # carries the vt chunks, so q rides the idle scalar queue there;
        # inside attention windows both sides use sync (gpsimd-triggered
        # DMAs proved slow to complete, stalling the next window's kT)
        q_eng = nc.scalar if prefix else nc.sync
        k_eng = nc.sync
        sides = (
            (qt_r, wq_sb, bq_sb, qTb[pc % 2], q_eng, streams, "xinl"),
            (kt_r, wk_sb, bk_sb, kTb[pc % 2], k_eng, streams2, "xinh"),
        )
        xins = {}
        order = [(sd, si) for si in range(NQ) for sd in range(2)]

        def fetch():
            if not order:
                return
            sd, si = order.pop(0)
            src_r, _, _, _, dma_eng, pool, tg = sides[sd]
            xin = pool.tile([P, NKC, 512], BF16, tag=tg)
            dma_eng.dma_start(out=xin, in_=src_r[:, :, ts(si, 512)])
            xins[(sd, si)] = xin

        fetch()
        fetch()
        yield
        fetch()
        fetch()
        yield
        yield
        for si in range(NQ):
            for side in range(2):
                _, w_sb, bias_sb, dst, _, _, _ = sides[side]
                fetch()
                ps = ps_proj.tile([P, 512], F32, tag="proj", name="psp")
                for kc in range(NKC):
                    nc.tensor.matmul(
                        ps,
                        lhsT=w_sb[:, kc, :],
                        rhs=xins[(side, si)][:, kc, :],
                        start=(kc == 0),
                        stop=(kc == NKC - 1),
                    )
                    if kc % 2 == 1:
                        yield
                del xins[(side, si)]
                nc.vector.tensor_add(
                    out=dst[si],
                    in0=ps,
                    in1=bias_sb[:, pc : pc + 1].to_broadcast([P, 512]),
                )
                yield

    def gen_oproj_block(blk):
        """O-projection for s-chunks 4*blk..4*blk+3 (needs all aoT at them)."""
        for si16 in range(4 * blk, 4 * blk + 4):
            for n2 in range(2):
                ps = ps_proj.tile([P, 512], F32, tag="proj", name="pso")
                for c in range(NPC):
                    nc.tensor.matmul(
                        ps,
                        lhsT=aoT[c][si16 // 4][:, ts(si16 % 4, P)],
                        rhs=wo_sb[:, c, ts(n2, 512)],
                        start=(c == 0),
                        stop=(c == NPC - 1),
                    )
                    if c % 2 == 1:
                        yield
                osb = outp.tile([P, 512], F32, tag="osb")
                nc.vector.tensor_copy(out=osb, in_=ps)
                nc.sync.dma_start(out=out_r[:, si16, ts(n2, 512)], in_=osb)
                yield

    pump = deque()

    def pump_n(n):
        k = 0
        while k < n and pump:
            try:
                next(pump[0])
                k += 1
            except StopIteration:
                pump.popleft()

    # ---- prefix: V projection (all heads) + qk projection for pc0 ----------
    for g in range(NQ):
        if g == 0:
            # first chunk in two one-shot halves so the kc=0..3 matmuls
            # start while the upper half (and everything else) streams in
            vin_q = []
            for q in range(2):
                t = vstream.tile([P, 4, 512], BF16, tag=f"v0h{q}", bufs=1)
                nc.sync.dma_start(out=t, in_=vt_r[:, 4 * q : 4 * q + 4, ts(g, 512)])
                vin_q.append(t)
        else:
            t = vstream.tile([P, NKC, 512], BF16, tag="vin")
            nc.sync.dma_start(out=t, in_=vt_r[:, :, ts(g, 512)])
            vin_q = [t[:, 0:4, :], t[:, 4:8, :]]
        for j in range(4):
            si16 = g * 4 + j
            ps = ps_proj.tile([P, 512], F32, tag="proj", name="psv")
            for kc in range(NKC):
                nc.tensor.matmul(
                    ps,
                    lhsT=vin_q[kc // 4][:, kc % 4, ts(j, P)],
                    rhs=wv_h[kc // 4][:, kc % 4, :],
                    start=(kc == 0),
                    stop=(kc == NKC - 1),
                )
            nc.vector.tensor_add(
                out=v_sb[si16][:, :, 0:64],
                in0=ps.rearrange("p (h d) -> p h d", h=8),
                in1=bv_rep.rearrange("p (h d) -> p h d", h=8),
            )
    for _ in gen_qk_proj(0, prefix=True):
        pass

    # ---- attention: per head pair, with pipelined filler --------------------
    for pc in range(NPC):
        hh = 2 * pc
        qT = qTb[pc % 2]
        kT = kTb[pc % 2]
        if pc < NPC - 1:
            pump.append(gen_qk_proj(pc + 1))
        for qi in range(NQ):
            av_e = ps_av.tile([P, 512], F32, tag="av", name="av_e")
            av_o = ps_av.tile([P, 512], F32, tag="av", name="av_o")
            for ki in range(NKI):
                st = ps_st.tile([P, 1024], F32, tag="st", name="st")
                kchunk = kT[ki // 4]
                nc.tensor.matmul(
                    st[:, 0:512],
                    lhsT=kchunk[0:64, ts(ki % 4, P)],
                    rhs=qT[qi][0:64, :],
                    start=True,
                    stop=True,
                )
                nc.tensor.matmul(
                    st[:, 512:1024],
                    lhsT=kchunk[64:128, ts(ki % 4, P)],
                    rhs=qT[qi][64:128, :],
                    start=True,
                    stop=True,
                    skip_group_check=True,
                )
                et = etp.tile([P, 1024], BF16, tag="et", name="et")
                nc.scalar.activation(out=et, in_=st, func=FP.Exp, scale=0.125)
                first = ki == 0
                last = ki == NKI - 1
                nc.tensor.matmul(
                    av_e[0:65],
                    lhsT=v_sb[ki][:, hh, 0:65],
                    rhs=et[:, 0:512],
                    start=first,
                    stop=last,
                    skip_group_check=True,
                )
                nc.tensor.matmul(
                    av_o[0:65],
                    lhsT=v_sb[ki][:, hh + 1, 0:65],
                    rhs=et[:, 512:1024],
                    start=first,
                    stop=last,
                    skip_group_check=True,
                )
                # front-load the qk projection filler: a dense PE queue keeps
                # the tensor engine in its max DVFS state (sparse filler
                # measurably drops it to the mid p-state, costing 2x on every
                # matmul); in the last head pair pump the O-projection only
                # once the previous qi's normalization chain has had time
                if pc < NPC - 1:
                    pump_n(3)
                elif ki >= 8:
                    pump_n(4)
            # normalization: copy PSUM out fast, reciprocal of the denominator
            # rows (partition 64 of each AV result), GPSIMD partition-broadcast
            ae = avsb.tile([P, 512], F32, tag="ae", name="ae")
            ao = avsb.tile([P, 512], F32, tag="ae", name="ao")
            nc.vector.tensor_copy(out=ae[0:65], in_=av_e[0:65])
            nc.vector.tensor_copy(out=ao[0:65], in_=av_o[0:65])
            # both denominator rows into one reciprocal: o's row parked at
            # partition 96 of ae, one recip covers partitions 64..96 (cost is
            # free-size-driven), results land on partitions 0 (e) and 32 (o).
            # partition_broadcast needs its input at partition 0 / offset 0,
            # so o's reciprocal is re-homed with one more 1-partition copy.
            nc.vector.tensor_copy(out=ae[96:97, :], in_=ao[64:65, :])
            rden_e = denp.tile([33, 512], F32, tag="rden_e", name="rden_e")
            rden_o = denp.tile([1, 512], F32, tag="rden_o", name="rden_o")
            nc.vector.reciprocal(out=rden_e[0:33, :], in_=ae[64:97, :])
            nc.vector.tensor_copy(out=rden_o[0:1, :], in_=rden_e[32:33, :])
            rep_e = repp.tile([64, 512], F32, tag="rep_e", name="rep_e")
            rep_o = repp.tile([64, 512], F32, tag="rep_o", name="rep_o")
            nc.gpsimd.partition_broadcast(rep_e[0:64, :], rden_e[0:1, :])
            nc.gpsimd.partition_broadcast(rep_o[0:64, :], rden_o[0:1, :])
            nc.vector.tensor_mul(
                out=aoT[pc][qi][0:64, :], in0=ae[0:64], in1=rep_e[0:64, :]
            )
            stag = stagp.tile([64, 512], BF16, tag="stag", name="stag")
            nc.vector.tensor_mul(
                out=stag, in0=ao[0:64], in1=rep_o[0:64, :]
            )
            nc.sync.dma_start(out=aoT[pc][qi][64:128, :], in_=stag)
            if pc == NPC - 1 and qi >= 1:
                pump.append(gen_oproj_block(qi - 1))
    pump.append(gen_oproj_block(NQ - 1))
    while pump:
        pump_n(8)


def _build():
    global _cached_nc
    if _cached_nc is not None:
        return _cached_nc
    nc = bacc.Bacc("TRN2", target_bir_lowering=False, debug=False)
    io = {
        "qt": nc.dram_tensor("qt", [D, S], BF16, kind="ExternalInput"),
        "kt": nc.dram_tensor("kt", [D, S], BF16, kind="ExternalInput"),
        "vt": nc.dram_tensor("vt", [D, S], BF16, kind="ExternalInput"),
        "wqt": nc.dram_tensor("wqt", [D, DOUT], BF16, kind="ExternalInput"),
        "wkt": nc.dram_tensor("wkt", [D, DOUT], BF16, kind="ExternalInput"),
        "wvt": nc.dram_tensor("wvt", [D, DOUT], BF16, kind="ExternalInput"),
        "wot": nc.dram_tensor("wot", [DOUT, D], BF16, kind="ExternalInput"),
        "bq": nc.dram_tensor("bq", [DOUT], F32, kind="ExternalInput"),
        "bk": nc.dram_tensor("bk", [DOUT], F32, kind="ExternalInput"),
        "bv": nc.dram_tensor("bv", [DOUT], F32, kind="ExternalInput"),
        "out": nc.dram_tensor("out", [S, D], F32, kind="ExternalOutput"),
    }
    with tile.TileContext(nc) as tc:
        with ExitStack() as ctx:
            _emit(ctx, tc, io)
    nc.compile()
    _cached_nc = nc
    return nc


def make_in_maps(Q, K, V, Wq, bq, Wk, bk, Wv, bv, Wo):
    bf = lambda a: np.ascontiguousarray(np.asarray(a, np.float32)).astype(
        ml_dtypes.bfloat16
    )
    f = lambda a: np.ascontiguousarray(a, dtype=np.float32)
    in_maps = []
    for c in range(N_CORES):
        b = c // 2
        lo = (c % 2) * DOUT
        sl = slice(lo, lo + DOUT)
        in_maps.append(
            {
                "qt": bf(np.asarray(Q, np.float32)[b].T),
                "kt": bf(np.asarray(K, np.float32)[b].T),
                "vt": bf(np.asarray(V, np.float32)[b].T),
                "wqt": bf(np.asarray(Wq, np.float32)[sl, :].T),
                "wkt": bf(np.asarray(Wk, np.float32)[sl, :].T),
                "wvt": bf(np.asarray(Wv, np.float32)[sl, :].T),
                "wot": bf(np.asarray(Wo, np.float32)[:, sl].T),
                "bq": f(bq[sl]),
                "bk": f(bk[sl]),
                "bv": f(bv[sl]),
            }
        )
    return in_maps


def gather_output(results, bo):
    out = np.empty((B, S, D), dtype=np.float32)
    bo = np.asarray(bo, dtype=np.float32)
    for b in range(B):
        out[b] = results[2 * b]["out"] + results[2 * b + 1]["out"] + bo
    return out


def _numpy_fallback(Q, K, V, mask, Wq, bq, Wk, bk, Wv, bv, Wo, bo):
    """Exact reference math in numpy (only used if mask isn't all-ones)."""
    H, dk = 16, 64
    out = np.empty((B, S, D), dtype=np.float32)
    for b in range(B):
        q = (Q[b] @ Wq.T + bq).reshape(S, H, dk).transpose(1, 0, 2)
        k = (K[b] @ Wk.T + bk).reshape(S, H, dk).transpose(1, 0, 2)
        v = (V[b] @ Wv.T + bv).reshape(S, H, dk).transpose(1, 0, 2)
        o = np.empty((H, S, dk), dtype=np.float32)
        for h in range(H):
            s = (q[h] @ k[h].T) / np.sqrt(np.float32(dk))
            s = np.where(mask[b] == 0, np.float32(-1.0e9), s)
            s = s - s.max(axis=-1, keepdims=True)
            e = np.exp(s)
            a = e / e.sum(axis=-1, keepdims=True)
            o[h] = a @ v[h]
        out[b] = o.transpose(1, 0, 2).reshape(S, H * dk) @ Wo.T + bo
    return out


def kernel(Q, K, V, mask, Wq, bq, Wk, bk, Wv, bv, Wo, bo):
    Q = np.asarray(Q, dtype=np.float32)
    K = np.asarray(K, dtype=np.float32)
    V = np.asarray(V, dtype=np.float32)
    Wq = np.asarray(Wq, dtype=np.float32)
    Wk = np.asarray(Wk, dtype=np.float32)
    Wv = np.asarray(Wv, dtype=np.float32)
    Wo = np.asarray(Wo, dtype=np.float32)
    bq = np.asarray(bq, dtype=np.float32)
    bk = np.asarray(bk, dtype=np.float32)
    bv = np.asarray(bv, dtype=np.float32)
    bo = np.asarray(bo, dtype=np.float32)
    mask_np = np.asarray(mask)

    if not np.all(mask_np != 0):
        return _numpy_fallback(Q, K, V, mask_np, Wq, bq, Wk, bk, Wv, bv, Wo, bo)

    nc = _build()
    in_maps = make_in_maps(Q, K, V, Wq, bq, Wk, bk, Wv, bv, Wo)
    res = run_bass_kernel_spmd(nc, in_maps, list(range(N_CORES))).results
    return gather_output(res, bo)


# revision 52
# speedup vs baseline: 1.0406x; 1.0406x over previous
"""Multi-head attention (B=4, S=2048, D=1024, H=16) on 8 Trainium2 cores.

Sharding: data-parallel over batch (4) x tensor-parallel over heads (2).
Core c handles batch c//2 and heads (c%2)*8 .. +8.  Each core computes a
partial output (its heads' contribution through the O-projection); the host
sums the two partials per batch and adds the output bias.

v3 schedule: the attention inner loop is scalar-engine-bound (exp), so all
other PE work is software-pipelined into it.  Q/K are projected one
head-pair at a time, with head-pair pc+1's projection matmuls interleaved
as filler into head-pair pc's attention window; the O-projection is
interleaved into the last head-pair's window.  Softmax normalization uses
same-partition copies of the in-AV denominator rows, one batched DVE
reciprocal, and GPSIMD partition-broadcasts (no DRAM round trips).
"""

import numpy as np
from collections import deque
from contextlib import ExitStack

import ml_dtypes
import concourse.bass as bass
import concourse.tile as tile
from concourse import bacc, library_config, mybir
from concourse.bass import ts
from concourse.bass_utils import run_bass_kernel_spmd

P = 128
S = 2048          # sequence length
D = 1024          # model dim
DOUT = 512        # per-core projection width (8 heads x 64)
DK = 64           # head dim
B = 4
N_CORES = 8
F32 = mybir.dt.float32
BF16 = mybir.dt.bfloat16
FP = mybir.ActivationFunctionType

NKC = D // P      # 8 contraction chunks over model dim
NPC = DOUT // P   # 4 head pairs per core
NQ = S // 512     # 4 query chunks of 512
NKI = S // P      # 16 key chunks of 128

_cached_nc = None


def _emit(ctx: ExitStack, tc: "tile.TileContext", io: dict):
    nc = tc.nc

    qt_r = io["qt"].ap().rearrange("(c p) s -> p c s", p=P)      # [128, 8, 2048]
    kt_r = io["kt"].ap().rearrange("(c p) s -> p c s", p=P)
    vt_r = io["vt"].ap().rearrange("(c p) s -> p c s", p=P)
    wqt_r = io["wqt"].ap().rearrange("(c p) m -> p c m", p=P)    # [128, 8, 512]
    wkt_r = io["wkt"].ap().rearrange("(c p) m -> p c m", p=P)
    wvt_r = io["wvt"].ap().rearrange("(c p) m -> p c m", p=P)
    wot_r = io["wot"].ap().rearrange("(c p) n -> p c n", p=P)    # [128, 4, 1024]
    bq_r = io["bq"].ap().rearrange("(c p) -> p c", p=P)          # [128, 4]
    bk_r = io["bk"].ap().rearrange("(c p) -> p c", p=P)
    bv_ap = io["bv"].ap()                                        # [512]
    out_r = io["out"].ap().rearrange("(sc p) n -> p sc n", p=P)  # [128, 16, 1024]

    persist = ctx.enter_context(tc.tile_pool(name="persist", bufs=1))
    weights = ctx.enter_context(tc.tile_pool(name="weights", bufs=2))
    streams = ctx.enter_context(tc.tile_pool(name="streams", bufs=4))
    streams2 = ctx.enter_context(tc.tile_pool(name="streams2", bufs=4))
    vstream = ctx.enter_context(tc.tile_pool(name="vstream", bufs=3))
    etp = ctx.enter_context(tc.tile_pool(name="etp", bufs=4))
    avsb = ctx.enter_context(tc.tile_pool(name="avsb", bufs=2))
    denp = ctx.enter_context(tc.tile_pool(name="denp", bufs=2))
    repp = ctx.enter_context(tc.tile_pool(name="repp", bufs=2))
    stagp = ctx.enter_context(tc.tile_pool(name="stagp", bufs=2))
    outp = ctx.enter_context(tc.tile_pool(name="outp", bufs=3))

    # PSUM: st 2x[128,1024] (4 banks) + av 2x[128,512] (2) + proj 2x[128,512] (2)
    ps_st = ctx.enter_context(tc.tile_pool(name="ps_st", bufs=2, space="PSUM"))
    ps_av = ctx.enter_context(tc.tile_pool(name="ps_av", bufs=2, space="PSUM"))
    ps_proj = ctx.enter_context(tc.tile_pool(name="ps_proj", bufs=2, space="PSUM"))

    # ---- constants / biases ------------------------------------------------
    # wv + first V input chunk gate the first matmul: wv rides the idle
    # scalar queue in parallel with vt chunks on sync, both split into
    # separate kc-half tiles (dependency tracking is tile-granular) so the
    # kc=0..3 matmuls can start while the second halves stream in
    wv_h = []
    for q in range(2):
        t = persist.tile([P, 4, DOUT], BF16, tag=f"wvh{q}")
        nc.scalar.dma_start(out=t, in_=wvt_r[:, 4 * q : 4 * q + 4, :])
        wv_h.append(t)
    # bv replicated across partitions (DMA partition-broadcast, stride 0)
    bv_rep = persist.tile([P, DOUT], F32, tag="bvrep")
    bv_bcast = bass.AP(
        tensor=bv_ap.tensor, offset=bv_ap.offset, ap=[[0, P]] + list(bv_ap.ap)
    )
    nc.gpsimd.dma_start(out=bv_rep, in_=bv_bcast)
    # gpsimd ucode library with InstPartitionBroadcast (standard lacks it);
    # emitted after the bv DMA so it doesn't gate the V projection
    nc.gpsimd.load_library(library_config.attn)
    bq_sb = persist.tile([P, NPC], F32, tag="bq")
    nc.gpsimd.dma_start(out=bq_sb, in_=bq_r)
    bk_sb = persist.tile([P, NPC], F32, tag="bk")
    nc.gpsimd.dma_start(out=bk_sb, in_=bk_r)
    wo_sb = persist.tile([P, NPC, D], BF16, tag="wo")
    nc.gpsimd.dma_start(out=wo_sb, in_=wot_r)

    # ---- persistent activations (bf16) --------------------------------------
    # qT/kT double-buffered across head pairs, split into four 512-col tiles
    # each: dependency tracking is tile-granular, so per-chunk tiles let the
    # next head pair's first scores start before its last chunk is projected
    qTb = [
        [
            persist.tile([P, 512], BF16, tag=f"qT{i}_{s}", name=f"qT{i}_{s}")
            for s in range(NQ)
        ]
        for i in range(2)
    ]
    kTb = [
        [
            persist.tile([P, 512], BF16, tag=f"kT{i}_{s}", name=f"kT{i}_{s}")
            for s in range(NQ)
        ]
        for i in range(2)
    ]
    # v: [s, head, dk+1]; col 64 of each head block holds ones so the AV
    # matmul's 65th output row accumulates the softmax denominator
    v_sb = [
        persist.tile([P, 8, 65], BF16, tag=f"v{i}", name=f"v{i}") for i in range(NKI)
    ]
    for i in range(NKI):
        nc.vector.memset(v_sb[i][:, :, 64:65], 1.0)
    # attn_outT: [dout, s] as 4x4 tiles of [128, 512] (rows 0-63 even head,
    # 64-127 odd; per-qi-block tiles keep O-projection dependencies exact)
    aoT = [
        [
            persist.tile([P, 512], BF16, tag=f"aoT{m}_{s}", name=f"aoT{m}_{s}")
            for s in range(NQ)
        ]
        for m in range(NPC)
    ]

    # ---- filler generators --------------------------------------------------
    def gen_qk_proj(pc, prefix=False):
        """Project qT/kT for head pair pc.  Each yield is ~1 PE slot."""
        wq_sb = weights.tile([P, NKC, P], BF16, tag="wq")
        nc.sync.dma_start(out=wq_sb, in_=wqt_r[:, :, ts(pc, P)])
        wk_sb = weights.tile([P, NKC, P], BF16, tag="wk")
        nc.sync.dma_start(out=wk_sb, in_=wkt_r[:, :, ts(pc, P)])
        yield
        # interleaved prefetch of input chunks; in the prefix sync still
        # carries the vt chunks, so q rides the idle scalar queue there;
        # inside attention windows both sides use sync (gpsimd-triggered
        # DMAs proved slow to complete, stalling the next window's kT)
        q_eng = nc.scalar if prefix else nc.sync
        k_eng = nc.sync
        sides = (
            (qt_r, wq_sb, bq_sb, qTb[pc % 2], q_eng, streams, "xinl"),
            (kt_r, wk_sb, bk_sb, kTb[pc % 2], k_eng, streams2, "xinh"),
        )
        xins = {}
        order = [(sd, si) for si in range(NQ) for sd in range(2)]

        def fetch():
            if not order:
                return
            sd, si = order.pop(0)
            src_r, _, _, _, dma_eng, pool, tg = sides[sd]
            xin = pool.tile([P, NKC, 512], BF16, tag=tg)
            dma_eng.dma_start(out=xin, in_=src_r[:, :, ts(si, 512)])
            xins[(sd, si)] = xin

        fetch()
        fetch()
        yield
        fetch()
        fetch()
        yield
        yield
        for si in range(NQ):
            for side in range(2):
                _, w_sb, bias_sb, dst, _, _, _ = sides[side]
                fetch()
                ps = ps_proj.tile([P, 512], F32, tag="proj", name="psp")
                for kc in range(NKC):
                    nc.tensor.matmul(
                        ps,
                        lhsT=w_sb[:, kc, :],
                        rhs=xins[(side, si)][:, kc, :],
                        start=(kc == 0),
                        stop=(kc == NKC - 1),
                    )
                    if kc % 2 == 1:
                        yield
                del xins[(side, si)]
                nc.vector.tensor_add(
                    out=dst[si],
                    in0=ps,
                    in1=bias_sb[:, pc : pc + 1].to_broadcast([P, 512]),
                )
                yield

    def gen_oproj_block(blk):
        """O-projection for s-chunks 4*blk..4*blk+3 (needs all aoT at them)."""
        for si16 in range(4 * blk, 4 * blk + 4):
            for n2 in range(2):
                ps = ps_proj.tile([P, 512], F32, tag="proj", name="pso")
                for c in range(NPC):
                    nc.tensor.matmul(
                        ps,
                        lhsT=aoT[c][si16 // 4][:, ts(si16 % 4, P)],
                        rhs=wo_sb[:, c, ts(n2, 512)],
                        start=(c == 0),
                        stop=(c == NPC - 1),
                    )
                    if c % 2 == 1:
                        yield
                osb = outp.tile([P, 512], F32, tag="osb")
                nc.vector.tensor_copy(out=osb, in_=ps)
                nc.sync.dma_start(out=out_r[:, si16, ts(n2, 512)], in_=osb)
                yield

    pump = deque()

    def pump_n(n):
        k = 0
        while k < n and pump:
            try:
                next(pump[0])
                k += 1
            except StopIteration:
                pump.popleft()

    # ---- prefix: V projection (all heads) + qk projection for pc0 ----------
    for g in range(NQ):
        if g == 0:
            # first chunk in two one-shot halves so the kc=0..3 matmuls
            # start while the upper half (and everything else) streams in
            vin_q = []
            for q in range(2):
                t = vstream.tile([P, 4, 512], BF16, tag=f"v0h{q}", bufs=1)
                nc.sync.dma_start(out=t, in_=vt_r[:, 4 * q : 4 * q + 4, ts(g, 512)])
                vin_q.append(t)
        else:
            t = vstream.tile([P, NKC, 512], BF16, tag="vin")
            nc.sync.dma_start(out=t, in_=vt_r[:, :, ts(g, 512)])
            vin_q = [t[:, 0:4, :], t[:, 4:8, :]]
        for j in range(4):
            si16 = g * 4 + j
            ps = ps_proj.tile([P, 512], F32, tag="proj", name="psv")
            for kc in range(NKC):
                nc.tensor.matmul(
                    ps,
                    lhsT=vin_q[kc // 4][:, kc % 4, ts(j, P)],
                    rhs=wv_h[kc // 4][:, kc % 4, :],
                    start=(kc == 0),
                    stop=(kc == NKC - 1),
                )
            nc.vector.tensor_add(
                out=v_sb[si16][:, :, 0:64],
                in0=ps.rearrange("p (h d) -> p h d", h=8),
                in1=bv_rep.rearrange("p (h d) -> p h d", h=8),
            )
    for _ in gen_qk_proj(0, prefix=True):
        pass

    # ---- attention: per head pair, with pipelined filler --------------------
    for pc in range(NPC):
        hh = 2 * pc
        qT = qTb[pc % 2]
        kT = kTb[pc % 2]
        if pc < NPC - 1:
            pump.append(gen_qk_proj(pc + 1))
        for qi in range(NQ):
            av_e = ps_av.tile([P, 512], F32, tag="av", name="av_e")
            av_o = ps_av.tile([P, 512], F32, tag="av", name="av_o")
            for ki in range(NKI):
                st = ps_st.tile([P, 1024], F32, tag="st", name="st")
                kchunk = kT[ki // 4]
                nc.tensor.matmul(
                    st[:, 0:512],
                    lhsT=kchunk[0:64, ts(ki % 4, P)],
                    rhs=qT[qi][0:64, :],
                    start=True,
                    stop=True,
                )
                nc.tensor.matmul(
                    st[:, 512:1024],
                    lhsT=kchunk[64:128, ts(ki % 4, P)],
                    rhs=qT[qi][64:128, :],
                    start=True,
                    stop=True,
                    skip_group_check=True,
                )
                et = etp.tile([P, 1024], BF16, tag="et", name="et")
                nc.scalar.activation(out=et, in_=st, func=FP.Exp, scale=0.125)
                first = ki == 0
                last = ki == NKI - 1
                nc.tensor.matmul(
                    av_e[0:65],
                    lhsT=v_sb[ki][:, hh, 0:65],
                    rhs=et[:, 0:512],
                    start=first,
                    stop=last,
                    skip_group_check=True,
                )
                nc.tensor.matmul(
                    av_o[0:65],
                    lhsT=v_sb[ki][:, hh + 1, 0:65],
                    rhs=et[:, 512:1024],
                    start=first,
                    stop=last,
                    skip_group_check=True,
                )
                # front-load the qk projection filler: a dense PE queue keeps
                # the tensor engine in its max DVFS state (sparse filler
                # measurably drops it to the mid p-state, costing 2x on every
                # matmul); in the last head pair pump the O-projection only
                # once the previous qi's normalization chain has had time
                if pc < NPC - 1:
                    pump_n(2)
                elif ki >= 8:
                    pump_n(4)
            # normalization: copy PSUM out fast, reciprocal of the denominator
            # rows (partition 64 of each AV result), GPSIMD partition-broadcast
            ae = avsb.tile([P, 512], F32, tag="ae", name="ae")
            ao = avsb.tile([P, 512], F32, tag="ae", name="ao")
            nc.vector.tensor_copy(out=ae[0:65], in_=av_e[0:65])
            nc.vector.tensor_copy(out=ao[0:65], in_=av_o[0:65])
            # both denominator rows into one reciprocal: o's row parked at
            # partition 96 of ae, one recip covers partitions 64..96 (cost is
            # free-size-driven), results land on partitions 0 (e) and 32 (o).
            # partition_broadcast needs its input at partition 0 / offset 0,
            # so o's reciprocal is re-homed with one more 1-partition copy.
            nc.vector.tensor_copy(out=ae[96:97, :], in_=ao[64:65, :])
            rden_e = denp.tile([33, 512], F32, tag="rden_e", name="rden_e")
            rden_o = denp.tile([1, 512], F32, tag="rden_o", name="rden_o")
            nc.vector.reciprocal(out=rden_e[0:33, :], in_=ae[64:97, :])
            nc.vector.tensor_copy(out=rden_o[0:1, :], in_=rden_e[32:33, :])
            rep_e = repp.tile([64, 512], F32, tag="rep_e", name="rep_e")
            rep_o = repp.tile([64, 512], F32, tag="rep_o", name="rep_o")
            nc.gpsimd.partition_broadcast(rep_e[0:64, :], rden_e[0:1, :])
            nc.gpsimd.partition_broadcast(rep_o[0:64, :], rden_o[0:1, :])
            nc.vector.tensor_mul(
                out=aoT[pc][qi][0:64, :], in0=ae[0:64], in1=rep_e[0:64, :]
            )
            stag = stagp.tile([64, 512], BF16, tag="stag", name="stag")
            nc.vector.tensor_mul(
                out=stag, in0=ao[0:64], in1=rep_o[0:64, :]
            )
            nc.sync.dma_start(out=aoT[pc][qi][64:128, :], in_=stag)
            if pc == NPC - 1 and qi >= 1:
                pump.append(gen_oproj_block(qi - 1))
    pump.append(gen_oproj_block(NQ - 1))
    while pump:
        pump_n(8)


def _build():
    global _cached_nc
    if _cached_nc is not None:
        return _cached_nc
    nc = bacc.Bacc("TRN2", target_bir_lowering=False, debug=False)
    io = {
        "qt": nc.dram_tensor("qt", [D, S], BF16, kind="ExternalInput"),
        "kt": nc.dram_tensor("kt", [D, S], BF16, kind="ExternalInput"),
        "vt": nc.dram_tensor("vt", [D, S], BF16, kind="ExternalInput"),
        "wqt": nc.dram_tensor("wqt", [D, DOUT], BF16, kind="ExternalInput"),
        "wkt": nc.dram_tensor("wkt", [D, DOUT], BF16, kind="ExternalInput"),
        "wvt": nc.dram_tensor("wvt", [D, DOUT], BF16, kind="ExternalInput"),
        "wot": nc.dram_tensor("wot", [DOUT, D], BF16, kind="ExternalInput"),
        "bq": nc.dram_tensor("bq", [DOUT], F32, kind="ExternalInput"),
        "bk": nc.dram_tensor("bk", [DOUT], F32, kind="ExternalInput"),
        "bv": nc.dram_tensor("bv", [DOUT], F32, kind="ExternalInput"),
        "out": nc.dram_tensor("out", [S, D], F32, kind="ExternalOutput"),
    }
    with tile.TileContext(nc) as tc:
        with ExitStack() as ctx:
            _emit(ctx, tc, io)
    nc.compile()
    _cached_nc = nc
    return nc


def make_in_maps(Q, K, V, Wq, bq, Wk, bk, Wv, bv, Wo):
    bf = lambda a: np.ascontiguousarray(np.asarray(a, np.float32)).astype(
        ml_dtypes.bfloat16
    )
    f = lambda a: np.ascontiguousarray(a, dtype=np.float32)
    in_maps = []
    for c in range(N_CORES):
        b = c // 2
        lo = (c % 2) * DOUT
        sl = slice(lo, lo + DOUT)
        in_maps.append(
            {
                "qt": bf(np.asarray(Q, np.float32)[b].T),
                "kt": bf(np.asarray(K, np.float32)[b].T),
                "vt": bf(np.asarray(V, np.float32)[b].T),
                "wqt": bf(np.asarray(Wq, np.float32)[sl, :].T),
                "wkt": bf(np.asarray(Wk, np.float32)[sl, :].T),
                "wvt": bf(np.asarray(Wv, np.float32)[sl, :].T),
                "wot": bf(np.asarray(Wo, np.float32)[:, sl].T),
                "bq": f(bq[sl]),
                "bk": f(bk[sl]),
                "bv": f(bv[sl]),
            }
        )
    return in_maps


def gather_output(results, bo):
    out = np.empty((B, S, D), dtype=np.float32)
    bo = np.asarray(bo, dtype=np.float32)
    for b in range(B):
        out[b] = results[2 * b]["out"] + results[2 * b + 1]["out"] + bo
    return out


def _numpy_fallback(Q, K, V, mask, Wq, bq, Wk, bk, Wv, bv, Wo, bo):
    """Exact reference math in numpy (only used if mask isn't all-ones)."""
    H, dk = 16, 64
    out = np.empty((B, S, D), dtype=np.float32)
    for b in range(B):
        q = (Q[b] @ Wq.T + bq).reshape(S, H, dk).transpose(1, 0, 2)
        k = (K[b] @ Wk.T + bk).reshape(S, H, dk).transpose(1, 0, 2)
        v = (V[b] @ Wv.T + bv).reshape(S, H, dk).transpose(1, 0, 2)
        o = np.empty((H, S, dk), dtype=np.float32)
        for h in range(H):
            s = (q[h] @ k[h].T) / np.sqrt(np.float32(dk))
            s = np.where(mask[b] == 0, np.float32(-1.0e9), s)
            s = s - s.max(axis=-1, keepdims=True)
            e = np.exp(s)
            a = e / e.sum(axis=-1, keepdims=True)
            o[h] = a @ v[h]
        out[b] = o.transpose(1, 0, 2).reshape(S, H * dk) @ Wo.T + bo
    return out


def kernel(Q, K, V, mask, Wq, bq, Wk, bk, Wv, bv, Wo, bo):
    Q = np.asarray(Q, dtype=np.float32)
    K = np.asarray(K, dtype=np.float32)
    V = np.asarray(V, dtype=np.float32)
    Wq = np.asarray(Wq, dtype=np.float32)
    Wk = np.asarray(Wk, dtype=np.float32)
    Wv = np.asarray(Wv, dtype=np.float32)
    Wo = np.asarray(Wo, dtype=np.float32)
    bq = np.asarray(bq, dtype=np.float32)
    bk = np.asarray(bk, dtype=np.float32)
    bv = np.asarray(bv, dtype=np.float32)
    bo = np.asarray(bo, dtype=np.float32)
    mask_np = np.asarray(mask)

    if not np.all(mask_np != 0):
        return _numpy_fallback(Q, K, V, mask_np, Wq, bq, Wk, bk, Wv, bv, Wo, bo)

    nc = _build()
    in_maps = make_in_maps(Q, K, V, Wq, bq, Wk, bk, Wv, bv, Wo)
    res = run_bass_kernel_spmd(nc, in_maps, list(range(N_CORES))).results
    return gather_output(res, bo)
